# revision 7
# baseline (speedup 1.0000x reference)
"""Trainium2 Bass kernel for the BidderStrategy MLP.

Math (per batch element x, per action n):
    out[n] = b2[n] + sum_h w2[n,h] * relu(w1[n,h] * x + b1[n,h])
    alpha[n] = w3[n, 0]

Since x = uniform[0,1), each hidden unit z = w1*x + b1 is monotone on the
input domain. Units that never cross zero are either always-off (dropped)
or always-linear (folded, in float64 on host, into a per-action affine
term a_n*x + c_n).  Only the crossing ("active") units are evaluated on
device.  This is a weight-only transformation - exact for every x in [0,1].

Device kernel (per core, batch-sharded 8 ways):
  - fc1: K=2 matmuls  psum_h = w1 (x) x + b1 (x) ones, row-tiled 4x on PE
  - relu: split between ACT (activation Relu) and DVE (tensor_scalar max)
  - fc2: K=128 matmuls accumulating into one PSUM bank; 4 batch subtiles
    are col-tiled into partition bands 32c..32c+12.  The affine fold rides
    rows 0/1 of j-tile 0 (x row and ones row of G0) with lhsT rows a_n,
    c_n + b2_n, so the bias needs no separate pass.
  - epilogue: one ACT copy PSUM->SBUF per 4 subtiles, then DMA out.
"""

import os

import numpy as np

NACT = 12
H = 200
B = 131072
NCORES = 8
B_CORE = B // NCORES  # 16384
BT = 512              # batch tile (matmul free dim, fp32 max)
NBT = B_CORE // BT    # 32
SG = 4                # batch subtiles sharing one output PSUM bank
NSG = NBT // SG       # 8

F32 = np.float32

# Filled by kernel() on each call: BassKernelResults of the last run.
LAST_RESULT = None


def _pack_weights(w1, b1, w2, b2):
    """Classify units and build the packed device constant arrays.

    Returns (T, w1b1_pack [128, T*128], a_pack [128, T*12], alin, clin).
    j-tile t uses PE row-group r = t % 4: w1 values sit in partition 32r,
    b1 values in partition 32r+1 of w1b1_pack.  a_pack row p, block t holds
    w2 of the unit at slot p of tile t (plus a/c' in rows 0/1 of tile 0).
    """
    w1f = w1[:, :, 0].astype(np.float64)   # [12, 200]
    b1f = b1.astype(np.float64)            # [12, 200]
    w2f = w2[:, 0, :].astype(np.float64)   # [12, 200]
    z0 = b1f
    z1 = w1f + b1f
    zero = np.maximum(z0, z1) <= 0
    linear = (np.minimum(z0, z1) >= 0) & ~zero
    active = ~zero & ~linear

    a_lin = (w2f * w1f * linear).sum(axis=1)                       # [12]
    c_lin = (w2f * b1f * linear).sum(axis=1) + b2[:, 0].astype(np.float64)

    acts = np.argwhere(active)             # [n_active, 2] (n, h)
    n_active = len(acts)
    # tile 0 rows 0/1 reserved for the x and ones rows
    T = 1 + max(0, -(-(n_active - 126) // 128))
    w1b1_pack = np.zeros((128, T * 128), dtype=F32)
    a_pack = np.zeros((128, T * 12), dtype=F32)

    w1_32 = w1[:, :, 0]  # float32 originals for exact device products
    slot = 2             # (tile 0, row 2) is the first unit slot
    for n, h in acts:
        t, p = divmod(slot, 128)
        r = t % 4
        w1b1_pack[32 * r + 0, t * 128 + p] = w1_32[n, h]
        w1b1_pack[32 * r + 1, t * 128 + p] = b1[n, h]
        a_pack[p, t * 12 + n] = w2[n, 0, h]
        slot += 1
    a_pack[0, 0 * 12: 1 * 12] = a_lin.astype(F32)
    a_pack[1, 0 * 12: 1 * 12] = c_lin.astype(F32)
    return T, w1b1_pack, a_pack


def _build_bass(T, w1b1_pack, a_pack, w3col):
    import concourse.bass as bass
    import concourse.mybir as mybir
    import concourse.tile as tile
    from concourse import bacc

    f32 = mybir.dt.float32
    nc = bacc.Bacc("TRN2", target_bir_lowering=False, debug=False)

    inp_d = nc.dram_tensor("inp", [B_CORE, 1], f32, kind="ExternalInput")
    out_d = nc.dram_tensor("out", [NACT, B_CORE], f32, kind="ExternalOutput")
    alpha_d = nc.dram_tensor("alpha", [1, NACT], f32, kind="ExternalOutput")

    w1b1_d = nc.inline_tensor(w1b1_pack, name="w1b1c")
    a_d = nc.inline_tensor(a_pack, name="apackc")
    ones_d = nc.inline_tensor(np.ones((1, BT), dtype=F32), name="onesc")
    w3_d = nc.inline_tensor(w3col.reshape(1, NACT), name="w3c")

    inp_flat = inp_d[:].rearrange("b one -> one b")  # [1, B_CORE]

    # relu engine per j-tile: True -> ACT, False -> DVE
    relu_on_act = [t % 2 == 0 for t in range(T)]

    with tile.TileContext(nc) as tc:
        with (
            tc.tile_pool(name="consts", bufs=1) as consts,
            tc.tile_pool(name="hdrp", bufs=3) as hdrp,
            tc.tile_pool(name="gp", bufs=8) as gp,
            tc.tile_pool(name="outp", bufs=2) as outp,
            tc.tile_pool(name="php", bufs=6, space="PSUM") as php,
            tc.tile_pool(name="pop", bufs=2, space="PSUM") as pop,
        ):
            w1b1_sb = consts.tile([128, T * 128], f32)
            nc.sync.dma_start(out=w1b1_sb[:], in_=w1b1_d[:])
            a_sb = consts.tile([128, T * 12], f32)
            nc.sync.dma_start(out=a_sb[:], in_=a_d[:])

            al_sb = consts.tile([1, NACT], f32)
            nc.sync.dma_start(out=al_sb[:], in_=w3_d[:])
            nc.sync.dma_start(out=alpha_d[:], in_=al_sb[:])

            for sg in range(NSG):
                po = pop.tile([128, BT], f32)
                for c in range(SG):
                    bt = sg * SG + c
                    hdr = hdrp.tile([128, BT], f32)
                    # stride between partitions, in elements
                    fs = hdr.tensor.shape[-1]
                    # x -> partitions {0,32,64,96}; ones -> {1,33,65,97}
                    hdr_x = bass.AP(hdr.tensor, 0, [[32 * fs, 4], [1, BT]])
                    hdr_1 = bass.AP(hdr.tensor, fs, [[32 * fs, 4], [1, BT]])
                    nc.sync.dma_start(
                        out=hdr_x,
                        in_=bass.AP(inp_d, bt * BT, [[0, 4], [1, BT]]),
                    )
                    nc.sync.dma_start(
                        out=hdr_1,
                        in_=bass.AP(ones_d, 0, [[0, 4], [1, BT]]),
                    )
                    for t in range(T):
                        r = t % 4
                        ph = php.tile([128, BT], f32)
                        nc.tensor.matmul(
                            ph[:],
                            w1b1_sb[32 * r: 32 * r + 2, 128 * t: 128 * (t + 1)],
                            hdr[32 * r: 32 * r + 2, :],
                            start=True,
                            stop=True,
                            tile_position=(32 * r, 0),
                        )
                        g = gp.tile([128, BT], f32)
                        if relu_on_act[t]:
                            nc.scalar.activation(
                                g[:], ph[:], mybir.ActivationFunctionType.Relu
                            )
                        else:
                            nc.vector.tensor_scalar_max(g[:], ph[:], 0.0)
                        if t == 0:
                            # overwrite rows 0 (x) and 1 (ones) after the relu;
                            # rows 2..31 keep the relu zeros (their w1b1 cols
                            # are zero), matching a_pack's zero rows there.
                            nc.sync.dma_start(
                                out=g[0:1, :], in_=inp_flat[:, bt * BT: (bt + 1) * BT]
                            )
                            nc.sync.dma_start(out=g[1:2, :], in_=ones_d[:])
                        nc.tensor.matmul(
                            po[32 * c: 32 * c + NACT, :],
                            a_sb[:, t * 12: (t + 1) * 12],
                            g[:],
                            start=(t == 0),
                            stop=(t == T - 1),
                            tile_position=(0, 32 * c),
                        )
                osb = outp.tile([128, BT], f32)
                nc.scalar.copy(osb[:], po[:])
                for c in range(SG):
                    bt = sg * SG + c
                    nc.sync.dma_start(
                        out=out_d[:, bt * BT: (bt + 1) * BT],
                        in_=osb[32 * c: 32 * c + NACT, :],
                    )

    nc.compile()
    return nc


def kernel(inp, w1, b1, w2, b2, w3):
    global LAST_RESULT
    from concourse.bass_utils import run_bass_kernel_spmd

    inp = np.ascontiguousarray(np.asarray(inp, dtype=F32))
    w1 = np.asarray(w1, dtype=F32)
    b1 = np.asarray(b1, dtype=F32)
    w2 = np.asarray(w2, dtype=F32)
    b2 = np.asarray(b2, dtype=F32)
    w3 = np.asarray(w3, dtype=F32)

    T, w1b1_pack, a_pack = _pack_weights(w1, b1, w2, b2)
    nc = _build_bass(T, w1b1_pack, a_pack, w3[:, 0].copy())

    in_maps = [
        {"inp": inp[i * B_CORE: (i + 1) * B_CORE]} for i in range(NCORES)
    ]
    trace = bool(int(os.environ.get("KERNEL_TRACE", "0")))
    res = run_bass_kernel_spmd(
        nc, in_maps, core_ids=list(range(NCORES)), trace=trace
    )
    LAST_RESULT = res

    out = np.concatenate([res.results[i]["out"] for i in range(NCORES)], axis=1)
    alpha = res.results[0]["alpha"].reshape(NACT).astype(F32)
    return alpha, out


# revision 8
# speedup vs baseline: 2.9343x; 2.9343x over previous
"""Trainium2 Bass kernel for the BidderStrategy MLP.

Math (per batch element x, per action n):
    out[n] = b2[n] + sum_h w2[n,h] * relu(w1[n,h] * x + b1[n,h])
    alpha[n] = w3[n, 0]

Since x = uniform[0,1), each hidden unit z = w1*x + b1 is monotone on the
input domain.  Units that never cross zero are either always-off (dropped)
or always-linear (folded, in float64 on host, into a per-action affine
term a_n*x + c_n).  Only the crossing ("active") units are evaluated on
device.  This is a weight-only transformation - exact for every x in [0,1].

Device kernel (per core, batch-sharded 8 ways, batch tiles of 512):
  - broadcast: one DMA replicates the x tile across all 128 partitions
  - fc1+relu: one fused op per j-tile of 128 active units:
      ACT: activation(Relu, scale=w1[p], bias=b1[p])  (1 instruction)
      DVE: tensor_scalar(mult w1[p], add b1[p]) + tensor_scalar_max
           (2 instructions, both at the 2x_2P fp32 SBUF perf mode)
  - fc2: K=128 fp32 matmuls; the 4 batch subtiles of a supergroup are
    col-tiled into partition bands 32c..32c+12 of ONE psum bank and run
    concurrently on the PE.  The affine fold rides rows 0/1 of j-tile 0
    (x row / ones row, DMA-overwritten after the relu) with lhsT rows
    a_n, c_n + b2_n, so the bias needs no separate pass.  j-tile 0 is
    consumed LAST so the row DMAs sit off the critical path.
  - epilogue: one ACT copy PSUM->SBUF per supergroup, then 4 DMAs out.
"""

import os

import numpy as np

NACT = 12
H = 200
B = 131072
NCORES = 8
B_CORE = B // NCORES  # 16384
BT = 512              # batch tile (matmul free dim, fp32 max)
NBT = B_CORE // BT    # 32
SG = 4                # batch subtiles sharing one output PSUM bank
NSG = NBT // SG       # 8

F32 = np.float32

# Filled by kernel() on each call: BassKernelResults of the last run.
LAST_RESULT = None


def _pack_weights(w1, b1, w2, b2):
    """Classify units and build packed device constants.

    Returns (T, w1_pack [128, T], b1_pack [128, T], a_pack [128, T*12]).
    j-tile t holds up to 128 active units; w1_pack[:, t] / b1_pack[:, t]
    are its per-partition scale/bias vectors.  a_pack[p, t*12+n] is the w2
    of the unit at slot p of tile t (0 elsewhere); rows 0/1 of tile 0 hold
    the folded linear term a_n and c_n + b2_n instead.
    """
    w1f = w1[:, :, 0].astype(np.float64)   # [12, 200]
    b1f = b1.astype(np.float64)            # [12, 200]
    w2f = w2[:, 0, :].astype(np.float64)   # [12, 200]
    z0 = b1f
    z1 = w1f + b1f
    zero = np.maximum(z0, z1) <= 0
    linear = (np.minimum(z0, z1) >= 0) & ~zero
    active = ~zero & ~linear

    a_lin = (w2f * w1f * linear).sum(axis=1)                       # [12]
    c_lin = (w2f * b1f * linear).sum(axis=1) + b2[:, 0].astype(np.float64)

    acts = np.argwhere(active)             # [n_active, 2] (n, h)
    n_active = len(acts)
    # tile 0 rows 0/1 reserved for the x and ones rows
    T = 1 + max(0, -(-(n_active - 126) // 128))
    w1_pack = np.zeros((128, T), dtype=F32)
    b1_pack = np.zeros((128, T), dtype=F32)
    a_pack = np.zeros((128, T * 12), dtype=F32)

    w1_32 = w1[:, :, 0]  # float32 originals for exact device products
    slot = 2             # (tile 0, row 2) is the first unit slot
    for n, h in acts:
        t, p = divmod(slot, 128)
        w1_pack[p, t] = w1_32[n, h]
        b1_pack[p, t] = b1[n, h]
        a_pack[p, t * 12 + n] = w2[n, 0, h]
        slot += 1
    a_pack[0, 0: NACT] = a_lin.astype(F32)
    a_pack[1, 0: NACT] = c_lin.astype(F32)
    return T, w1_pack, b1_pack, a_pack


def _build_bass(T, w1_pack, b1_pack, a_pack, w3col):
    import concourse.bass as bass
    import concourse.mybir as mybir
    import concourse.tile as tile
    from concourse import bacc

    f32 = mybir.dt.float32
    Relu = mybir.ActivationFunctionType.Relu
    mult = mybir.AluOpType.mult
    add = mybir.AluOpType.add
    nc = bacc.Bacc("TRN2", target_bir_lowering=False, debug=False)

    inp_d = nc.dram_tensor("inp", [B_CORE, 1], f32, kind="ExternalInput")
    out_d = nc.dram_tensor("out", [NACT, B_CORE], f32, kind="ExternalOutput")
    alpha_d = nc.dram_tensor("alpha", [1, NACT], f32, kind="ExternalOutput")

    w1_d = nc.inline_tensor(w1_pack, name="w1c")
    b1_d = nc.inline_tensor(b1_pack, name="b1c")
    a_d = nc.inline_tensor(a_pack, name="apackc")
    ones_d = nc.inline_tensor(np.ones((1, BT), dtype=F32), name="onesc")
    w3_d = nc.inline_tensor(w3col.reshape(1, NACT), name="w3c")

    inp_flat = inp_d[:].rearrange("b one -> one b")  # [1, B_CORE]

    # fc2 consumption order: j-tile 0 last, so its post-relu x/ones row
    # DMAs get ~6 matmul rounds of slack.
    mm_order = list(range(1, T)) + [0]

    with tile.TileContext(nc) as tc:
        with (
            tc.tile_pool(name="consts", bufs=1) as consts,
            tc.tile_pool(name="xbp", bufs=6) as xbp,
            tc.tile_pool(name="gp", bufs=12) as gp,
            tc.tile_pool(name="outp", bufs=2) as outp,
            tc.tile_pool(name="pop", bufs=2, space="PSUM") as pop,
        ):
            w1_sb = consts.tile([128, T], f32)
            nc.sync.dma_start(out=w1_sb[:], in_=w1_d[:])
            b1_sb = consts.tile([128, T], f32)
            nc.sync.dma_start(out=b1_sb[:], in_=b1_d[:])
            a_sb = consts.tile([128, T * 12], f32)
            nc.sync.dma_start(out=a_sb[:], in_=a_d[:])

            al_sb = consts.tile([1, NACT], f32)
            nc.sync.dma_start(out=al_sb[:], in_=w3_d[:])
            nc.sync.dma_start(out=alpha_d[:], in_=al_sb[:])

            for sg in range(NSG):
                po = pop.tile([128, BT], f32)
                xbs = []
                gs = []
                for c in range(SG):
                    bt = sg * SG + c
                    xb = xbp.tile([128, BT], f32)
                    # replicate the 512-element x tile into all partitions
                    nc.sync.dma_start(
                        out=xb[:],
                        in_=bass.AP(inp_d, bt * BT, [[0, 128], [1, BT]]),
                    )
                    xbs.append(xb)
                    gs.append([None] * T)
                # fused fc1+relu, interleaved across ACT and DVE
                for t in range(T):
                    for c in range(SG):
                        g = gp.tile([128, BT], f32)
                        gs[c][t] = g
                        w1ap = w1_sb[:, t: t + 1]
                        b1ap = b1_sb[:, t: t + 1]
                        if (t * SG + c) % 2 == 0:
                            nc.scalar.activation(
                                g[:], xbs[c][:], Relu, bias=b1ap, scale=w1ap
                            )
                        else:
                            nc.vector.tensor_scalar(
                                g[:], xbs[c][:], w1ap, b1ap, mult, add
                            )
                            nc.vector.tensor_scalar_max(g[:], g[:], 0.0)
                        if t == 0:
                            bt = sg * SG + c
                            nc.sync.dma_start(
                                out=g[0:1, :],
                                in_=inp_flat[:, bt * BT: (bt + 1) * BT],
                            )
                            nc.sync.dma_start(out=g[1:2, :], in_=ones_d[:])
                # fc2: col-tiled accumulation, 4 subtiles concurrent
                for ti, t in enumerate(mm_order):
                    for c in range(SG):
                        nc.tensor.matmul(
                            po[32 * c: 32 * c + NACT, :],
                            a_sb[:, t * 12: (t + 1) * 12],
                            gs[c][t][:],
                            start=(ti == 0),
                            stop=(ti == T - 1),
                            tile_position=(0, 32 * c),
                        )
                osb = outp.tile([128, BT], f32)
                nc.scalar.copy(osb[:], po[:])
                for c in range(SG):
                    bt = sg * SG + c
                    nc.sync.dma_start(
                        out=out_d[:, bt * BT: (bt + 1) * BT],
                        in_=osb[32 * c: 32 * c + NACT, :],
                    )

    nc.compile()
    return nc


def kernel(inp, w1, b1, w2, b2, w3):
    global LAST_RESULT
    from concourse.bass_utils import run_bass_kernel_spmd

    inp = np.ascontiguousarray(np.asarray(inp, dtype=F32))
    w1 = np.asarray(w1, dtype=F32)
    b1 = np.asarray(b1, dtype=F32)
    w2 = np.asarray(w2, dtype=F32)
    b2 = np.asarray(b2, dtype=F32)
    w3 = np.asarray(w3, dtype=F32)

    T, w1_pack, b1_pack, a_pack = _pack_weights(w1, b1, w2, b2)
    nc = _build_bass(T, w1_pack, b1_pack, a_pack, w3[:, 0].copy())

    in_maps = [
        {"inp": inp[i * B_CORE: (i + 1) * B_CORE]} for i in range(NCORES)
    ]
    trace = bool(int(os.environ.get("KERNEL_TRACE", "0")))
    res = run_bass_kernel_spmd(
        nc, in_maps, core_ids=list(range(NCORES)), trace=trace
    )
    LAST_RESULT = res

    out = np.concatenate([res.results[i]["out"] for i in range(NCORES)], axis=1)
    alpha = res.results[0]["alpha"].reshape(NACT).astype(F32)
    return alpha, out
